# revision 6
# baseline (speedup 1.0000x reference)
"""Two-layer GAT (PyG-style GATConv x2) on 8 Trainium2 NeuronCores — v3.

No device-side dma_gather (SWDGE Q7 descriptor generation was the v1
bottleneck at ~5.3ns/row). The host marshals per-edge source feature rows
into dst-sorted streams between launches; the device consumes them with
plain sequential HWDGE DMA and does all the math (matmuls, softmax, segment
reduction). The host performs only data movement (indexed replication /
layout) plus parameter-space weight folding (W0@A0); every arithmetic op on
activations stays on device.

Edge layout per core: edges are partitioned by dst shard, grouped per
128-row dst tile. Within tile t, the first nident_t edges of every dst d go
to "identity chunks" (partition = d, lhsT = I, no one-hot needed);
leftovers go to "one-hot chunks" (lhsT built on-chip from a streamed
dst-in-tile row via is_equal). Chunk counts are per-tile (max over the 8
cores, SPMD-uniform). Each 128-edge chunk is segment-reduced into the
tile's PSUM by one matmul whose rhs is the softmax-weighted feature block.

Feature rows are head-interleaved ([65 feat x H heads], feat 64 = the ones
column that becomes the softmax denominator after weighting) so the
per-edge weight multiply runs in the DVE 2x_1P perf mode. A fraction of the
weight multiplies and the cheap adds run on GpSimd to balance engines.

Launches: 1) h0 = x @ [W0 | W0@A0]; 2) layer-0 edges + ELU + h1 = z @ W1;
3) layer-1 edges + bias -> output.
"""

import os

import numpy as np

import concourse.bacc as bacc
import concourse.mybir as mybir
from concourse import tile
from concourse.bass_utils import run_bass_kernel_spmd

fp32 = mybir.dt.float32
bf16 = mybir.dt.bfloat16
Alu = mybir.AluOpType
Act = mybir.ActivationFunctionType

NCORES = 8
NEG_SLOPE = 0.2
EPS = 1e-16
# Padded slots are harmless with any finite alpha: their G rows are all-zero
# (including the ones column), so both the message and the denominator
# contribution are w * 0 = 0.
PAD_ALPHA = 0.0

N = 50000
NLOC = 6250
NP = 6272  # padded to mult of 128
NT = NP // 128
F_IN = 256
HID = 256
H0_HEADS = 4
C_OUT = 64
CALLC = 32  # chunks per stream DMA call


# ---------------------------------------------------------------- launch 1


def build_l1():
    """h0 = x_shard @ W0ext -> hT (bf16 col-major) + alphas ([8,NP] fp32)."""
    nc = bacc.Bacc(None, target_bir_lowering=False, debug=False)
    xTb = nc.dram_tensor("xTb", [F_IN, NP], bf16, kind="ExternalInput")
    W0b = nc.dram_tensor("W0b", [F_IN, HID], bf16, kind="ExternalInput")
    WAb = nc.dram_tensor("WAb", [F_IN, 8], bf16, kind="ExternalInput")
    hT = nc.dram_tensor("hT", [HID, NP], bf16, kind="ExternalOutput")
    a0o = nc.dram_tensor("a0o", [8, NP], fp32, kind="ExternalOutput")

    TW = 1024
    n_t = (NP + TW - 1) // TW

    with tile.TileContext(nc) as tc:
        with (
            tc.tile_pool(name="const", bufs=1) as cpool,
            tc.tile_pool(name="work", bufs=3) as pool,
            tc.tile_pool(name="psum", bufs=2, space="PSUM") as pp,
            tc.tile_pool(name="psum1", bufs=2, space="PSUM") as pp1,
        ):
            w0_sb = [cpool.tile([128, HID], bf16, tag=f"w0_{k}", name=f"w0_{k}") for k in range(2)]
            wa_sb = [cpool.tile([128, 8], bf16, tag=f"wa_{k}", name=f"wa_{k}") for k in range(2)]
            for k in range(2):
                nc.sync.dma_start(w0_sb[k][:], W0b[128 * k : 128 * (k + 1), :])
                nc.sync.dma_start(wa_sb[k][:], WAb[128 * k : 128 * (k + 1), :])

            for t in range(n_t):
                c0 = t * TW
                cw = min(TW, NP - c0)
                xt = [pool.tile([128, TW], bf16, tag=f"xt{k}", name=f"xt{k}") for k in range(2)]
                for k in range(2):
                    nc.sync.dma_start(
                        xt[k][:, :cw], xTb[128 * k : 128 * (k + 1), c0 : c0 + cw]
                    )
                nw = (cw + 511) // 512
                for w in range(nw):
                    w0 = w * 512
                    ww = min(512, cw - w0)
                    for m in range(2):
                        ps = pp.tile([128, 512], fp32, tag=f"ps{m}", name=f"ps{m}")
                        for k in range(2):
                            nc.tensor.matmul(
                                ps[:, :ww],
                                w0_sb[k][:, 128 * m : 128 * (m + 1)],
                                xt[k][:, w0 : w0 + ww],
                                start=(k == 0),
                                stop=(k == 1),
                            )
                        hb = pool.tile([128, 512], bf16, tag=f"hb{m}", name=f"hb{m}")
                        nc.vector.tensor_copy(hb[:, :ww], ps[:, :ww])
                        nc.sync.dma_start(
                            hT[128 * m : 128 * (m + 1), c0 + w0 : c0 + w0 + ww],
                            hb[:, :ww],
                        )
                    pa = pp1.tile([8, 512], fp32, tag="pa")
                    for k in range(2):
                        nc.tensor.matmul(
                            pa[:, :ww],
                            wa_sb[k][:],
                            xt[k][:, w0 : w0 + ww],
                            start=(k == 0),
                            stop=(k == 1),
                        )
                    av = pool.tile([8, 512], fp32, tag="av")
                    nc.vector.tensor_copy(av[:, :ww], pa[:, :ww])
                    nc.sync.dma_start(a0o[:, c0 + w0 : c0 + w0 + ww], av[:, :ww])
    nc.compile()
    return nc


# ------------------------------------------------------------ edge machinery


def _edge_pass(nc, tc, d, Gs, ABs, OHs, eyeb, nheads, nfeat, fin, mul_gps, iota=None, pair=False, callc=CALLC, ohcall=8, gbufs=4):
    """Stream dst-sorted weighted edges; segment-reduce per dst tile.

    mul_gps: fraction of call-multiplies routed to GpSimd."""
    NIDENT_T, NOH_T = d["NIDENT_T"], d["NOH_T"]
    CH_OFF, OH_OFF = d["CH_OFF"], d["OH_OFF"]
    NCHTOT, NOHTOT = d["NCHTOT"], d["NOHTOT"]
    RW = nfeat * nheads

    with (
        tc.tile_pool(name="econst", bufs=1) as ipool,
        tc.tile_pool(name="edge", bufs=4) as pool,
        tc.tile_pool(name="epsum", bufs=3, space="PSUM") as pp,
    ):
        eye_sb = ipool.tile([128, 128], bf16, tag="eyeb")
        nc.sync.dma_start(eye_sb[:], eyeb[:])
        oh_stream = iota is None
        if not oh_stream:
            iota_sb = ipool.tile([128, 128], bf16, tag="iotas")
            nc.sync.dma_start(iota_sb[:], iota[:])
            rr_sb = ipool.tile([128, NOHTOT], bf16, tag="rrs")
            nc.sync.dma_start(rr_sb[:], OHs[:])

        tiles = {}
        ohtiles = {}
        state = dict(ncalls=0, nmul=0, nohcalls=0)
        OHCALL = ohcall

        # in pair mode, "lanes" plays the role of heads for the weighting:
        # a call covers CALLC chunks = CALLC//2 pairs of 2 lanes each.
        lanes = 2 if pair else nheads
        CU = callc // 2 if pair else callc
        NU = (NCHTOT + 1) // 2 if pair else NCHTOT

        def emit_call(q):
            c0 = q * CU
            nch = min(CU, NU - c0)
            G = pool.tile([128, CU, nfeat * lanes], bf16, tag="G", name="G", bufs=gbufs)
            AB = pool.tile([128, CU, 2 * lanes], fp32, tag="AB", name="AB", bufs=gbufs)
            nc.sync.dma_start(G[:, :nch, :], Gs[:, c0 : c0 + nch, :])
            nc.sync.dma_start(AB[:, :nch, :], ABs[:, c0 : c0 + nch, :])
            ew = pool.tile([128, CU, lanes], fp32, tag="ew", name="ew", bufs=gbufs)
            nc.vector.tensor_tensor(
                ew[:, :nch, :],
                AB[:, :nch, 0:lanes],
                AB[:, :nch, lanes : 2 * lanes],
                op=Alu.add,
            )
            nc.vector.scalar_tensor_tensor(
                ew[:, :nch, :],
                ew[:, :nch, :],
                NEG_SLOPE,
                ew[:, :nch, :],
                op0=Alu.mult,
                op1=Alu.max,
            )
            ewb = pool.tile([128, CU, lanes], bf16, tag="ewb", name="ewb", bufs=gbufs)
            nc.scalar.activation(ewb[:, :nch, :], ew[:, :nch, :], Act.Exp)
            if lanes > 1:
                gv = G[:, :nch, :].rearrange("p c (f h) -> p c f h", h=lanes)
                wv = (
                    ewb[:, :nch, :]
                    .unsqueeze(2)
                    .broadcast_to([128, nch, nfeat, lanes])
                )
            else:
                gv = G[:, :nch, :]
                wv = ewb[:, :nch, :].broadcast_to([128, nch, nfeat])
            use_gps = int((state["nmul"] + 1) * mul_gps) > int(state["nmul"] * mul_gps)
            state["nmul"] += 1
            (nc.gpsimd if use_gps else nc.vector).tensor_tensor(
                gv, gv, wv, op=Alu.mult
            )
            return G

        def get_chunk(c):
            q = c // callc
            while state["ncalls"] <= q:
                tiles[state["ncalls"]] = emit_call(state["ncalls"])
                state["ncalls"] += 1
                tiles.pop(state["ncalls"] - gbufs, None)
            k = c % callc
            G = tiles[q]
            if pair:
                return G, G[:, k // 2, :].rearrange("p (f s) -> p f s", s=2)[:, :, k % 2]
            return G, G[:, k, 0:RW]

        def get_oh(o):
            if not oh_stream:
                OH = pool.tile([128, 128], bf16, tag="OH", name="OH", bufs=4)
                rrv = rr_sb[:, o].unsqueeze(1).broadcast_to([128, 128])
                nc.vector.tensor_tensor(OH[:], rrv, iota_sb[:], op=Alu.is_equal)
                return OH, None
            q = o // OHCALL
            while state["nohcalls"] <= q:
                qq = state["nohcalls"]
                o0 = qq * OHCALL
                n = min(OHCALL, NOHTOT - o0)
                OH = pool.tile([128, OHCALL, 128], bf16, tag="OH", name="OH", bufs=4)
                nc.sync.dma_start(OH[:, :n, :], OHs[:, o0 : o0 + n, :])
                ohtiles[qq] = OH
                state["nohcalls"] += 1
                ohtiles.pop(qq - 4, None)
            return ohtiles[q], o % OHCALL

        for t in range(NT):
            nident, noh = NIDENT_T[t], NOH_T[t]
            nch_t = nident + noh
            ps = pp.tile([128, RW], fp32, tag="ps")
            for j in range(nch_t):
                G, rhs = get_chunk(CH_OFF[t] + j)
                if j < nident:
                    lhsT = eye_sb[:]
                else:
                    OH, ko = get_oh(OH_OFF[t] + (j - nident))
                    lhsT = OH[:] if ko is None else OH[:, ko, :]
                nc.tensor.matmul(
                    ps[:],
                    lhsT,
                    rhs,
                    start=(j == 0),
                    stop=(j == nch_t - 1),
                )
            fin(t, ps)


# ---------------------------------------------------------------- launch 2


def build_l2(d):
    """Layer-0 edge pass + fused finalize (softmax-div, bias, ELU) +
    h1 = z @ W1 (+alphas) -> h1T bf16 + a1 ([2,NP] fp32)."""
    nc = bacc.Bacc(None, target_bir_lowering=False, debug=False)
    NCHTOT = d["NCHTOT"]
    RW = 65 * H0_HEADS

    Gs = nc.dram_tensor("Gs", [128, NCHTOT, RW], bf16, kind="ExternalInput")
    ABs = nc.dram_tensor("ABs", [128, NCHTOT, 8], fp32, kind="ExternalInput")
    OHs = nc.dram_tensor("OHs", [128, d["NOHTOT"], 128], bf16, kind="ExternalInput")
    eyeb = nc.dram_tensor("eyeb", [128, 128], bf16, kind="ExternalInput")
    W1 = nc.dram_tensor("W1", [HID, C_OUT], fp32, kind="ExternalInput")
    A1 = nc.dram_tensor("A1", [C_OUT, 2], fp32, kind="ExternalInput")
    b0r = nc.dram_tensor("b0r", [128, HID], fp32, kind="ExternalInput")
    eye = nc.dram_tensor("eye", [128, 128], fp32, kind="ExternalInput")
    if d.get("LDW_OPT"):
        nc.dram_tensor("ldwopt", [1, 2], fp32, kind="ExternalInput")
    h1T = nc.dram_tensor("h1T", [C_OUT, NP], bf16, kind="ExternalOutput")
    a1o = nc.dram_tensor("a1o", [NP, 2], fp32, kind="ExternalOutput")

    with tile.TileContext(nc) as tc:
        with (
            tc.tile_pool(name="fconst", bufs=1) as cpool,
            tc.tile_pool(name="fin", bufs=4) as pool,
            tc.tile_pool(name="tb1psum", bufs=1, space="PSUM") as pp2,
        ):
            b0_sb = cpool.tile([128, HID], fp32)
            nc.sync.dma_start(b0_sb[:], b0r[:])
            eye_f = cpool.tile([128, 128], fp32, tag="eyef")
            nc.sync.dma_start(eye_f[:], eye[:])
            w1_sb = [cpool.tile([128, C_OUT], fp32, tag=f"w1_{k}", name=f"w1_{k}") for k in range(2)]
            for k in range(2):
                nc.sync.dma_start(w1_sb[k][:], W1[128 * k : 128 * (k + 1), :])
            a1_sb = cpool.tile([C_OUT, 2], fp32)
            nc.sync.dma_start(a1_sb[:], A1[:])

            def fin0(t, ps):
                dn = pool.tile([128, 4], fp32, tag="dn")
                nc.vector.tensor_scalar_add(dn[:], ps[:, 256:260], EPS)
                rec = pool.tile([128, 4], fp32, tag="rec")
                nc.vector.reciprocal(rec[:], dn[:])
                z = pool.tile([128, HID], fp32, tag="z")
                zv = z[:].rearrange("p (f h) -> p f h", h=4)
                pv = ps[:, 0:256].rearrange("p (f h) -> p f h", h=4)
                rb = rec[:].unsqueeze(1).broadcast_to([128, 64, 4])
                nc.vector.tensor_tensor(zv, pv, rb, op=Alu.mult)
                if not d["B0_ZERO"]:
                    nc.vector.tensor_tensor(z[:], z[:], b0_sb[:], op=Alu.add)
                # ELU = exp(min(z,0)) - 1 + relu(z)
                tn = pool.tile([128, HID], fp32, tag="tn")
                (nc.gpsimd if d["FIN_GPS"] else nc.vector).tensor_scalar_min(tn[:], z[:], 0.0)
                nc.scalar.activation(tn[:], tn[:], Act.Exp)
                tp = pool.tile([128, HID], fp32, tag="tp")
                nc.scalar.activation(tp[:], z[:], Act.Relu)
                nc.vector.scalar_tensor_tensor(
                    z[:], tn[:], -1.0, tp[:], op0=Alu.add, op1=Alu.add
                )
                # ---- table1: h1 = z @ W1, a1 = h1 @ A1
                h0T = [
                    pool.tile([128, 128], fp32, tag=f"h0T{k}", name=f"h0T{k}", bufs=3)
                    for k in range(2)
                ]
                for k in range(2):
                    pt = pp2.tile([128, 128], fp32, tag="pt", bufs=2)
                    nc.tensor.transpose(
                        pt[:], z[:, 128 * k : 128 * (k + 1)], eye_f[:]
                    )
                    nc.vector.tensor_copy(h0T[k][:], pt[:])
                ph1 = pp2.tile([C_OUT, 128], fp32, tag="ph1", bufs=2)
                for k in range(2):
                    nc.tensor.matmul(
                        ph1[:], w1_sb[k][:], h0T[k][:], start=(k == 0), stop=(k == 1)
                    )
                h1f = pool.tile([C_OUT, 128], fp32, tag="h1f")
                nc.vector.tensor_copy(h1f[:], ph1[:])
                h1b = pool.tile([C_OUT, 128], bf16, tag="h1b")
                nc.scalar.activation(h1b[:], ph1[:], Act.Copy)
                nc.sync.dma_start(h1T[:, 128 * t : 128 * (t + 1)], h1b[:])
                pal = pp2.tile([128, 2], fp32, tag="pal", bufs=1)
                nc.tensor.matmul(pal[:], h1f[:], a1_sb[:], start=True, stop=True)
                av = pool.tile([128, 2], fp32, tag="av")
                nc.vector.tensor_copy(av[:], pal[:])
                nc.sync.dma_start(a1o[128 * t : 128 * (t + 1), :], av[:])

            _edge_pass(
                nc, tc, d, Gs, ABs, OHs, eyeb, 4, 65, fin0, d["MUL_GPS2"],
                callc=48, ohcall=12, gbufs=5,
            )
    nc.compile()
    return nc


# ---------------------------------------------------------------- launch 3


def build_l3(d):
    """Layer-1 edge pass -> + bias -> output rows."""
    nc = bacc.Bacc(None, target_bir_lowering=False, debug=False)
    NCHTOT = d["NCHTOT"]
    RW = 65

    NPAIR = (NCHTOT + 1) // 2
    Gs = nc.dram_tensor("Gs", [128, NPAIR, 130], bf16, kind="ExternalInput")
    ABs = nc.dram_tensor("ABs", [128, NPAIR, 4], fp32, kind="ExternalInput")
    OHs = nc.dram_tensor("OHs", [128, d["NOHTOT"], 128], bf16, kind="ExternalInput")
    eyeb = nc.dram_tensor("eyeb", [128, 128], bf16, kind="ExternalInput")
    b1r = nc.dram_tensor("b1r", [128, C_OUT], fp32, kind="ExternalInput")
    out = nc.dram_tensor("out", [NP, C_OUT], fp32, kind="ExternalOutput")

    with tile.TileContext(nc) as tc:
        with (
            tc.tile_pool(name="oconst", bufs=1) as cpool,
            tc.tile_pool(name="ofin", bufs=4) as pool,
        ):
            b1_sb = cpool.tile([128, C_OUT], fp32)
            nc.sync.dma_start(b1_sb[:], b1r[:])

            def fin1(t, ps):
                dn = pool.tile([128, 1], fp32, tag="dn")
                nc.vector.tensor_scalar_add(dn[:], ps[:, 64:65], EPS)
                rec = pool.tile([128, 1], fp32, tag="rec")
                nc.vector.reciprocal(rec[:], dn[:])
                O = pool.tile([128, C_OUT], fp32, tag="O")
                rb = rec[:].broadcast_to([128, C_OUT])
                nc.vector.tensor_tensor(O[:], ps[:, 0:C_OUT], rb, op=Alu.mult)
                if not d["B1_ZERO"]:
                    nc.vector.tensor_tensor(O[:], O[:], b1_sb[:], op=Alu.add)
                nc.sync.dma_start(out[128 * t : 128 * (t + 1), :], O[:])

            _edge_pass(
                nc, tc, d, Gs, ABs, OHs, eyeb, 1, 65, fin1, d["MUL_GPS3"], pair=True,
                callc=64, ohcall=16, gbufs=6,
            )
    nc.compile()
    return nc


# ------------------------------------------------------------ host plumbing


def _bf16(a):
    import ml_dtypes

    return np.asarray(a).astype(ml_dtypes.bfloat16)


def _prep_edges(edge_index):
    """Slot edges per core: per-tile identity chunks (partition = dst-in-tile)
    for the first nident_t edges of each dst, leftovers packed into one-hot
    chunks. Per-tile counts are maxed over cores (SPMD uniformity)."""
    src = np.concatenate([edge_index[0], np.arange(N, dtype=np.int64)])
    dst = np.concatenate([edge_index[1], np.arange(N, dtype=np.int64)])
    core = dst // NLOC
    per_core = []
    cnts = np.zeros((NCORES, NT, 128), np.int64)
    for c in range(NCORES):
        m = core == c
        s, dl = src[m], dst[m] - c * NLOC
        order = np.argsort(dl, kind="stable")
        s, dl = s[order], dl[order]
        cnt = np.bincount(dl, minlength=NP)
        off = np.concatenate([[0], np.cumsum(cnt)])
        rank = np.arange(len(dl)) - off[dl]
        per_core.append((s, dl, rank))
        cnts[c] = cnt.reshape(NT, 128)

    # per-tile (q_t, noh_t): minimize chunks, tie -> larger q (less one-hot)
    NIDENT_T, NOH_T = [], []
    for t in range(NT):
        best = None
        for q in range(0, 33):
            lo = np.maximum(cnts[:, t, :] - q, 0).sum(axis=1)  # per core
            noh = int(np.ceil(lo.max() / 128)) if lo.max() > 0 else 0
            nch = max(q + noh, 1)
            if best is None or nch < best[0] or (nch == best[0] and q > best[1]):
                best = (nch, q, noh)
        NIDENT_T.append(best[1])
        NOH_T.append(best[2])
    CH_OFF = np.concatenate(
        [[0], np.cumsum([NIDENT_T[t] + NOH_T[t] for t in range(NT)])]
    ).tolist()
    OH_OFF = np.concatenate([[0], np.cumsum(NOH_T)]).tolist()
    NCHTOT, NOHTOT = CH_OFF[-1], max(OH_OFF[-1], 1)

    res = []
    for s, dl, rank in per_core:
        eids = np.full((NCHTOT, 128), -1, np.int64)
        rr = np.full((NOHTOT, 128), -1.0, np.float32)
        tt = dl // 128
        din = dl % 128
        nid_arr = np.asarray(NIDENT_T, np.int64)
        ch_arr = np.asarray(CH_OFF)
        oh_arr = np.asarray(OH_OFF)
        ident = rank < nid_arr[tt]
        eids[ch_arr[tt[ident]] + rank[ident], din[ident]] = np.nonzero(ident)[0]
        lm = ~ident
        lt = tt[lm]
        lidx = np.nonzero(lm)[0]
        tcnt = np.bincount(lt, minlength=NT)
        toff = np.concatenate([[0], np.cumsum(tcnt)])
        lpos = np.arange(len(lidx)) - toff[lt]
        ch = ch_arr[lt] + nid_arr[lt] + lpos // 128
        eids[ch, lpos % 128] = lidx
        oh = oh_arr[lt] + lpos // 128
        rr[oh, lpos % 128] = din[lm]
        res.append(dict(eids=eids, s=s, dl=dl, rr=rr))
    d = dict(
        NIDENT_T=NIDENT_T,
        NOH_T=NOH_T,
        CH_OFF=CH_OFF,
        OH_OFF=OH_OFF,
        NCHTOT=NCHTOT,
        NOHTOT=NOHTOT,
    )
    return d, res


def _streams(core_d, a_node, a_src_cols, a_dst_cols, feat_T, nheads, c):
    """Build Gs/AB for one core from node-level device outputs.

    feat_T: [F, N] bf16 col-major node features; a_node: [A, N] fp32 alphas
    (col-major). Gs [128, NCHTOT, (F//nheads+1)*nheads] head-interleaved with
    ones cols; AB [128, NCHTOT, 2*nheads] = [alpha_src | alpha_dst]."""
    eids = core_d["eids"]  # [NCHTOT, 128]
    s, dl = core_d["s"], core_d["dl"]
    NCHTOT = eids.shape[0]
    F = feat_T.shape[0]
    fe = F // nheads
    valid = eids >= 0
    e = np.where(valid, eids, 0)
    srcs = s[e]  # [NCHTOT, 128]
    dstg = dl[e] + c * NLOC
    rows = feat_T[:, srcs.ravel()].T  # [S, F] bf16
    S = rows.shape[0]
    G = np.zeros((S, (fe + 1) * nheads), rows.dtype)
    G[:, : fe * nheads] = (
        rows.reshape(S, nheads, fe).transpose(0, 2, 1).reshape(S, fe * nheads)
    )
    G[:, fe * nheads :] = _bf16(1.0)
    G[~valid.ravel()] = 0
    Gs = np.ascontiguousarray(G.reshape(NCHTOT, 128, -1).transpose(1, 0, 2))
    AB = np.empty((S, 2 * nheads), np.float32)
    AB[:, 0:nheads] = a_node[a_src_cols, :][:, srcs.ravel()].T
    AB[:, nheads:] = a_node[a_dst_cols, :][:, dstg.ravel()].T
    AB[~valid.ravel()] = PAD_ALPHA
    AB = np.ascontiguousarray(AB.reshape(NCHTOT, 128, -1).transpose(1, 0, 2))
    return Gs, AB




def _maybe_patch_ldw():
    """Optional experiment: let walrus dedup identical consecutive
    LoadStationary ops (identity-matrix chunks reload the same weights)."""
    if not int(os.environ.get("GAT_LDW_OPT", "0")):
        return False
    import concourse.bass_utils as bu

    if not getattr(bu, "_ldw_patched", False):
        orig = bu.run_command

        def patched(argv, **kw):
            argv = [
                "--enable-ldw-opt=true" if a == "--enable-ldw-opt=false" else a
                for a in argv
            ]
            return orig(argv, **kw)

        bu.run_command = patched
        bu._ldw_patched = True
    return True


def _pair_pack(Gs, AB):
    """Pack chunk pairs as 2 interleaved lanes: Gs [128,C,F] ->
    [128,ceil(C/2),2F] with col (f,s); AB [128,C,2] -> [128,ceil(C/2),4]."""
    C, F = Gs.shape[1], Gs.shape[2]
    NP2 = (C + 1) // 2
    if C % 2:
        Gs = np.concatenate([Gs, np.zeros((128, 1, F), Gs.dtype)], axis=1)
        AB = np.concatenate([AB, np.zeros((128, 1, 2), AB.dtype)], axis=1)
    Gp = np.ascontiguousarray(
        Gs.reshape(128, NP2, 2, F).transpose(0, 1, 3, 2).reshape(128, NP2, 2 * F)
    )
    ABp = np.ascontiguousarray(
        AB.reshape(128, NP2, 2, 2).transpose(0, 1, 3, 2).reshape(128, NP2, 4)
    )
    return Gp, ABp


def _oh_stream(rr):
    """One-hot bytes [128, NOHTOT, 128] bf16 from dst-in-tile rows
    rr [NOHTOT, 128] (-1 = padded slot -> zero row)."""
    NOHTOT = rr.shape[0]
    oh = np.zeros((128, NOHTOT, 128), np.float32)
    p, o = np.meshgrid(np.arange(128), np.arange(NOHTOT), indexing="ij")
    rrT = rr.T  # [128, NOHTOT]
    m = rrT >= 0
    oh[p[m], o[m], rrT[m].astype(np.int64)] = 1.0
    return _bf16(oh)


_cache = {}
LAST_PROFILE = {}


def _run(nc, in_maps, core_ids, label):
    import sys

    trace = bool(int(os.environ.get("GAT_PROFILE", "0")))
    if trace:
        try:
            import profile_hook

            profile_hook.install()
            import concourse.bass_utils as bu

            bu.upload_artifacts = lambda tmpdir: "local://skipped"
            br = run_bass_kernel_spmd(nc, in_maps, core_ids, trace=True)
            LAST_PROFILE[label] = br.exec_time_ns
            return br.results
        except Exception as e:  # fall back to untraced
            print(f"traced run failed ({e!r}); untraced retry", file=sys.stderr)
    br = run_bass_kernel_spmd(nc, in_maps, core_ids)
    LAST_PROFILE[label] = br.exec_time_ns
    return br.results


def kernel(x, edge_index, W0, att_src0, att_dst0, b0, W1, att_src1, att_dst1, b1):
    x = np.asarray(x, np.float32)
    edge_index = np.asarray(edge_index)
    d, cores = _prep_edges(edge_index)

    d["MUL_GPS2"] = float(os.environ.get("GAT_MUL_GPS2", "0.0"))
    d["MUL_GPS3"] = float(os.environ.get("GAT_MUL_GPS3", "0.0"))
    d["FIN_GPS"] = bool(int(os.environ.get("GAT_FIN_GPS", "0")))
    d["B0_ZERO"] = bool(np.all(np.asarray(b0) == 0))
    d["B1_ZERO"] = bool(np.all(np.asarray(b1) == 0))
    d["LDW_OPT"] = _maybe_patch_ldw()
    key = (d["NCHTOT"], tuple(d["NIDENT_T"]), tuple(d["NOH_T"]),
           d["MUL_GPS2"], d["MUL_GPS3"], d["FIN_GPS"], d["B0_ZERO"], d["B1_ZERO"],
           d["LDW_OPT"])
    if key not in _cache:
        _cache.clear()
        _cache[key] = (build_l1(), build_l2(d), build_l3(d))
    nc1, nc2, nc3 = _cache[key]

    # interleave map for layer-0 hidden dim: col f*4+h <- h*64+f
    il = (np.arange(64)[:, None] + 64 * np.arange(4)[None, :]).ravel()
    core_ids = list(range(NCORES))
    iota = _bf16(np.tile(np.arange(128, dtype=np.float32)[None, :], (128, 1)))
    eye = np.eye(128, dtype=np.float32)
    eyeb = _bf16(eye)

    # ---- launch 1: W0ext = [W0 | W0 @ A0-vectors]
    W0f = np.asarray(W0, np.float32)
    a_s0 = np.asarray(att_src0, np.float32)
    a_d0 = np.asarray(att_dst0, np.float32)
    WA = np.zeros((F_IN, 8), np.float32)
    for h in range(4):
        WA[:, h] = W0f[:, h * 64 : (h + 1) * 64] @ a_s0[h]
        WA[:, 4 + h] = W0f[:, h * 64 : (h + 1) * 64] @ a_d0[h]
    xb = _bf16(x)
    in1 = []
    for c in range(NCORES):
        xT = np.zeros((F_IN, NP), xb.dtype)
        xT[:, :NLOC] = xb[c * NLOC : (c + 1) * NLOC].T
        in1.append(dict(xTb=xT, W0b=_bf16(W0f), WAb=_bf16(WA)))
    r1 = _run(nc1, in1, core_ids, "l1")
    h0T = np.concatenate([r1[c]["hT"][:, :NLOC] for c in range(NCORES)], axis=1)
    a0 = np.concatenate([r1[c]["a0o"][:, :NLOC] for c in range(NCORES)], axis=1)

    # ---- launch 2
    b0r = np.tile(np.asarray(b0, np.float32)[il][None, :], (128, 1))
    A1 = np.stack(
        [np.asarray(att_src1).ravel(), np.asarray(att_dst1).ravel()], axis=1
    ).astype(np.float32)
    W1f = np.asarray(W1, np.float32)[il, :]  # permuted rows to match z layout
    in2 = []
    for c in range(NCORES):
        Gs, AB = _streams(cores[c], a0, [0, 1, 2, 3], [4, 5, 6, 7], h0T, 4, c)
        in2.append(
            dict(
                Gs=Gs,
                ABs=AB,
                OHs=_oh_stream(cores[c]["rr"]),
                eyeb=eyeb,
                W1=W1f,
                A1=A1,
                b0r=b0r,
                eye=eye,
                **({"ldwopt": np.zeros((1, 2), np.float32)} if d["LDW_OPT"] else {}),
            )
        )
    r2 = _run(nc2, in2, core_ids, "l2")
    h1T = np.concatenate([r2[c]["h1T"][:, :NLOC] for c in range(NCORES)], axis=1)
    a1 = np.concatenate(
        [r2[c]["a1o"][:NLOC] for c in range(NCORES)], axis=0
    ).T.copy()  # [2, N]

    # ---- launch 3
    b1r = np.tile(np.asarray(b1, np.float32)[None, :], (128, 1))
    in3 = []
    for c in range(NCORES):
        Gs, AB = _streams(cores[c], a1, [0], [1], h1T, 1, c)
        Gs, AB = _pair_pack(Gs, AB)
        in3.append(dict(Gs=Gs, ABs=AB, OHs=_oh_stream(cores[c]["rr"]), eyeb=eyeb, b1r=b1r))
    r3 = _run(nc3, in3, core_ids, "l3")
    out = np.concatenate([r3[c]["out"][:NLOC] for c in range(NCORES)], axis=0)
    return out
